# revision 22
# baseline (speedup 1.0000x reference)
"""Trainium2 8-core kernel for nn_Attention_34402688041077.

Reference computation (fp32):
    qkv = x @ W_qkv.T + b_qkv          x:[2,2048,1024], W_qkv:[3072,1024]
    q,k,v per head (H=16, HD=64)
    attn = softmax(q k^T / sqrt(64)); out = attn v
    y = out @ W_proj.T + b_proj

Sharding (tensor parallel over heads):
  - core c computes heads {2c, 2c+1} for the whole batch: QKV projection
    (column-sliced W_qkv), attention, then contributes its [128, 4096]
    slice of the pre-projection activations (feature-major layout) to an
    AllGather (split into 5 token-block collectives so the projection
    overlaps attention of later blocks and the serial tail is short).
  - output projection is sharded over output features: core c computes
    y[:, 128c:128c+128] for all 4096 tokens from the gathered [1024, *].
  - host assembles y from the 8 feature slices.

Schedule: the tensor engine is the binding resource (sum of matmul
moving-columns ~165us vs ~134us of scalar-engine exp), so the body is
one continuous attention pipeline (128 score-tiles of [128 ktok, 1024
q]) with every other PE op (QKV chunks, projection pieces, v
transposes) issued as "filler" units inside the exp-wait slack via a
build-time credit scheduler.  attn@v lags exp by LAG tiles so v-chunk
fillers can complete in time.  Across reps the pipeline is software
pipelined: rep r's last projections and rep r+1's batch-0 QKV fill the
neighbouring rep's attention slack (pools are hoisted out of the body
so no drain barriers separate reps).

Numerics: matmul inputs in bf16 (fp32 PSUM accumulation), softmax exp in
fp32 on the scalar engine without max-subtraction (scores are ~N(0,1),
|s|<10, exp cannot overflow), attention row sums via an appended
ones-column on v so they fall out of the attn@v matmul, normalization as
reciprocal-multiply after the matmul.  v-bias folds through softmax into
the projection bias (host-side).  Measured rel err vs fp32 ref ~5e-3.

All host work is reshapes/transposes/dtype casts; every FLOP runs on the
NeuronCores.
"""

import sys

sys.path.insert(0, "/opt/trn_rl_repo")

import numpy as np
import ml_dtypes

import concourse.bass as bass  # noqa: F401  (registers engine types)
import concourse.tile as tile
from concourse import bacc, mybir
from concourse.bass_utils import run_bass_kernel_spmd
from concourse.masks import make_identity

BF16_NP = ml_dtypes.bfloat16
F32 = mybir.dt.float32
BF16 = mybir.dt.bfloat16

N_CORES = 8
B, N, DIM, H, HD = 2, 2048, 1024, 16, 64
T = B * N                # 4096 flattened tokens
HPC = H // N_CORES       # 2 heads per core
FPC = HPC * HD           # 128 features per core
SCALE = 1.0 / np.sqrt(HD)

LAG = 4                  # attn@v tiles behind exp tiles
RATE = 450.0             # filler ns pumped per score-tile iteration
MM = 213.0               # one 512-row bf16 matmul
TP = 53.0                # one 128x128 transpose

# gather/projection blocks: (batch, col0, width)
GBLK = [(0, 0, 1024), (0, 1024, 1024), (1, 0, 1024),
        (1, 1024, 512), (1, 1536, 512)]

_NC_CACHE = {}


def _mm(nc, out, lhsT, rhs, start, stop):
    """matmul with the moving/output free dim split to 512 (PSUM-bank limit
    for fp32 accumulation)."""
    n = rhs.shape[-1]
    for o in range(0, n, 512):
        w = min(512, n - o)
        nc.tensor.matmul(out[:, o:o + w], lhsT=lhsT, rhs=rhs[:, o:o + w],
                         start=start, stop=stop)


class _Sched:
    """Build-time credit scheduler: filler generators yield the PE cost of
    the instruction(s) they just emitted; pump(ns) advances the queue by
    that much filler budget.  WINDOW=1 (strictly sequential generators) so
    per-tag PSUM rotation is single-owner and deadlock-free."""

    def __init__(self):
        self.q = []
        self.cur = None      # (name, gen)
        self.credit = 0.0
        self.done = set()

    def add(self, name, gen):
        self.q.append((name, gen))

    def _step(self):
        if self.cur is None:
            if not self.q:
                return False
            self.cur = self.q.pop(0)
        name, gen = self.cur
        try:
            self.credit -= next(gen)
        except StopIteration:
            self.done.add(name)
            self.cur = None
        return True

    def pump(self, ns):
        self.credit = min(self.credit + ns, 4000.0)
        while self.credit > 0:
            if not self._step():
                self.credit = 0.0
                return

    def drain(self, name):
        if name in self.done:
            return
        saved = self.credit
        while name not in self.done:
            if not self._step():
                raise RuntimeError(f"drain({name}): queue empty")
        self.credit = saved

    def drain_all(self):
        while self._step():
            pass


def _nm_qk(r, bb, ft, tcb):
    return f"qk{r}_{bb}_{ft}{tcb}"


def _nm_v(r, bb, tcb):
    return f"v{r}_{bb}{tcb}"


class _Ctx:
    """Everything the emitters need: nc, pools, slabs, weights, I/O."""

    def __init__(self, nc, collective):
        self.nc = nc
        self.collective = collective


def _emit_body(nc, tc, C, reps):
    """Emit the full rep-pipelined program (pools already set up in C)."""
    EXP = mybir.ActivationFunctionType.Exp
    sch = C.sch

    # ---------------- filler generators ----------------
    def gen_qk(r, bb, ft, tcb):
        dst = (C.qsl if ft == 0 else C.ksl)[bb]
        ps = C.psF.tile([128, 512], F32, tag="fill", name="psqk")
        for kc in range(8):
            nc.tensor.matmul(
                ps[:], lhsT=C.w_sb[kc][:, ft * 128:(ft + 1) * 128],
                rhs=C.xT_sb[kc][bb][:, tcb * 512:(tcb + 1) * 512],
                start=(kc == 0), stop=(kc == 7))
            yield MM
        nc.vector.tensor_scalar_add(
            dst[:, tcb * 512:(tcb + 1) * 512], ps[:], C.bqk_sb[:, ft:ft + 1])
        yield 0.0

    def gen_v(r, bb, tcb):
        # v computed transposed (weight-stationary), PE-transposed back into
        # v_ext's [tok%128, tok_tile, head, HD+1] layout
        ps = C.psF.tile([128, 512], F32, tag="fill", name="psvt")
        for kc in range(8):
            nc.tensor.matmul(
                ps[:], lhsT=C.w_sb[kc][:, 256:384],
                rhs=C.xT_sb[kc][bb][:, tcb * 512:(tcb + 1) * 512],
                start=(kc == 0), stop=(kc == 7))
            yield MM
        vt = C.attnp.tile([128, 512], BF16, tag="vt", name="vt")
        nc.vector.tensor_copy(vt[:], ps[:])
        yield 800.0  # DVE copy latency before the first transpose can run
        for jj in range(4):
            a = tcb * 4 + jj
            tp = C.psF.tile([128, 128], BF16, tag="fill", name="tp")
            nc.tensor.transpose(tp[:], vt[:, jj * 128:(jj + 1) * 128],
                                C.ident[:])
            nc.vector.tensor_copy(C.v_ext[bb][:, a, 0, 0:HD], tp[:, 0:HD])
            nc.vector.tensor_copy(C.v_ext[bb][:, a, 1, 0:HD],
                                  tp[:, HD:2 * HD])
            yield TP + 100.0

    def gen_proj(r, tb):
        bb, col0, w = GBLK[tb]
        rts = []
        for p0 in range(0, w, 512):
            rt = C.rhp.tile([128, 8, 512], BF16, tag="agr", name="agr")
            nc.sync.dma_start(
                out=rt[:],
                in_=C.ag_out[r][tb][:, p0:p0 + 512].rearrange(
                    "(j p) t -> p j t", p=128))
            rts.append(rt)
        yield 900.0  # let the first rhs DMA land before the first matmul
        for pi, p0 in enumerate(range(0, w, 512)):
            ps = C.psF.tile([128, 512], F32, tag="fill", name="psp")
            for j in range(8):
                nc.tensor.matmul(
                    ps[:], lhsT=C.wp_sb[j][:], rhs=rts[pi][:, j, :],
                    start=(j == 0), stop=(j == 7))
                yield MM
            ysb = C.yp.tile([128, 512], F32, tag="ysb", name="ysb")
            nc.vector.tensor_scalar_add(ysb[:], ps[:], C.bp_sb[:])
            c = bb * N + col0 + p0
            nc.sync.dma_start(out=C.y[:, c:c + 512], in_=ysb[:])
            yield 0.0

    def add_qkv_batch(r, bb):
        for tcb in range(4):
            sch.add(_nm_qk(r, bb, 1, tcb), gen_qk(r, bb, 1, tcb))
        for tcb in range(4):
            sch.add(_nm_qk(r, bb, 0, tcb), gen_qk(r, bb, 0, tcb))
        for tcb in range(4):
            sch.add(_nm_v(r, bb, tcb), gen_v(r, bb, tcb))

    # ---------------- attention ----------------
    def attn_block(r, bb, h, qb, appends, bi, next_q=None):
        colq = qb * 1024
        # two half-width accumulators so the next block's first half can
        # start accumulating while this block's second half still normalizes
        ao_h = [C.psAO.tile([HD + 1, 512], F32, tag=f"ao{i}", name=f"ao{i}")
                for i in range(2)]
        ats = {}

        def av(j):
            sch.drain(_nm_v(r, bb, j // 4))
            at = ats.pop(j)
            for i in range(2):
                nc.tensor.matmul(
                    ao_h[i][:], lhsT=C.v_ext[bb][:, j, h, :],
                    rhs=at[:, i * 512:(i + 1) * 512],
                    start=(j == 0), stop=(j == 15))

        for kc in range(16):
            for nm, g in appends.pop((bi, kc), []):
                sch.add(nm, g)
            if kc == 0:
                sch.drain(_nm_qk(r, bb, 0, 2 * qb))
                sch.drain(_nm_qk(r, bb, 0, 2 * qb + 1))
            if kc % 4 == 0:
                sch.drain(_nm_qk(r, bb, 1, kc // 4))
            # prefetch upcoming k/v chunks (and the next block's q) a couple
            # of iterations early so their DVE copy/bias chains settle
            # before the PE consumes them
            if kc + 2 <= 15:
                sch.drain(_nm_qk(r, bb, 1, (kc + 2) // 4))
            j2 = kc - LAG + 2
            if 0 <= j2 <= 15:
                sch.drain(_nm_v(r, bb, j2 // 4))
            if kc == 14 and next_q is not None:
                for nm in next_q:
                    sch.drain(nm)
            colk = kc * 128
            sc = C.psSC.tile([128, 1024], F32, tag="sc", name="sc")
            _mm(nc, sc,
                lhsT=C.ksl[bb][h * HD:(h + 1) * HD, colk:colk + 128],
                rhs=C.qsl[bb][h * HD:(h + 1) * HD, colq:colq + 1024],
                start=True, stop=True)
            at = C.attnp.tile([128, 1024], BF16, tag="at", name="at")
            nc.scalar.activation(out=at[:], in_=sc[:], func=EXP)
            ats[kc] = at
            if kc - LAG >= 0:
                av(kc - LAG)
            sch.pump(RATE)
        for j in range(16 - LAG, 16):
            av(j)
            sch.pump(300.0)
        # normalize from SBUF in DVE slack; sums row staged through
        # partition 0 for partition_broadcast (gpsimd, off critical path)
        for o in (0, 512):
            ar = C.aoraw.tile([HD + 1, 512], F32, tag="ar", name="ar")
            nc.vector.tensor_copy(ar[:], ao_h[o // 512][:])
            srow = C.normp.tile([1, 512], F32, tag="srow", name="srow")
            nc.gpsimd.tensor_copy(srow[:], ar[HD:HD + 1, :])
            bc = C.normp.tile([HD, 512], F32, tag="bc", name="bc")
            nc.gpsimd.partition_broadcast(bc[:], srow[:])
            rec = C.normp.tile([HD, 512], F32, tag="rec", name="rec")
            nc.vector.reciprocal(rec[:], bc[:])
            nc.vector.tensor_mul(
                C.aosl[bb][h][:, colq + o:colq + o + 512],
                ar[0:HD, :], rec[:])
            sch.pump(300.0)

    def stage(r, tb, h):
        # stage this head's slice to the bounce buffer as soon as its norm
        # lands (fast HWDGE queue, off the collective's critical path)
        bb, col0, w = GBLK[tb]
        nc.sync.dma_start(
            out=C.ag_in[r][tb][h * HD:(h + 1) * HD, :],
            in_=C.aosl[bb][h][:, col0:col0 + w])

    def collect(r, tb):
        if C.collective:
            nc.gpsimd.collective_compute(
                "AllGather", mybir.AluOpType.bypass,
                replica_groups=[list(range(N_CORES))],
                ins=[C.ag_in[r][tb][:].opt()], outs=[C.ag_out[r][tb][:].opt()],
            )
        else:  # timing-sim variant: token dep so proj waits on attn
            nc.gpsimd.dma_start(out=C.ag_out[r][tb][0:1, 0:128],
                                in_=C.ag_in[r][tb][0:1, 0:128])

    # ---------------- cold prologue (rep 0, batch 0 q/k over half the
    # tokens, kc-outer so QKV streams against the x DMAs) ----------------
    cold = [
        ("q", 0, C.psSC.tile([128, 512], F32, tag="sc", name="cq0")),
        ("q", 1, C.psSC.tile([128, 512], F32, tag="sc", name="cq1")),
        ("k", 0, C.psAO.tile([128, 512], F32, tag="ao0", name="ck0")),
        ("k", 1, C.psF.tile([128, 512], F32, tag="fill", name="ck1")),
    ]
    for kc in range(8):
        for ft, (_, tcb, ps) in enumerate(cold):
            f = 0 if ft < 2 else 1
            nc.tensor.matmul(
                ps[:], lhsT=C.w_sb[kc][:, f * 128:(f + 1) * 128],
                rhs=C.xT_sb[kc][0][:, tcb * 512:(tcb + 1) * 512],
                start=(kc == 0), stop=(kc == 7))
    for ft, (_, tcb, ps) in enumerate(cold):
        f = 0 if ft < 2 else 1
        dst = (C.qsl if f == 0 else C.ksl)[0]
        nc.vector.tensor_scalar_add(
            dst[:, tcb * 512:(tcb + 1) * 512], ps[:], C.bqk_sb[:, f:f + 1])
    sch.done.update({_nm_qk(0, 0, 0, 0), _nm_qk(0, 0, 0, 1),
                     _nm_qk(0, 0, 1, 0), _nm_qk(0, 0, 1, 1)})

    # ---------------- rep pipeline ----------------
    for r in range(reps):
        C.alloc_ag(r)
        if r == 0:
            # cold leftovers, ordered by first use
            sch.add(_nm_v(0, 0, 0), gen_v(0, 0, 0))
            sch.add(_nm_v(0, 0, 1), gen_v(0, 0, 1))
            sch.add(_nm_qk(0, 0, 1, 2), gen_qk(0, 0, 1, 2))
            sch.add(_nm_qk(0, 0, 1, 3), gen_qk(0, 0, 1, 3))
            sch.add(_nm_v(0, 0, 2), gen_v(0, 0, 2))
            sch.add(_nm_v(0, 0, 3), gen_v(0, 0, 3))
            sch.add(_nm_qk(0, 0, 0, 2), gen_qk(0, 0, 0, 2))
            sch.add(_nm_qk(0, 0, 0, 3), gen_qk(0, 0, 0, 3))
        add_qkv_batch(r, 1)

        appends = {}
        if r > 0:
            appends[(0, 2)] = [(f"proj{r - 1}_3", gen_proj(r - 1, 3))]
            appends[(0, 8)] = [(f"proj{r - 1}_4", gen_proj(r - 1, 4))]
        appends[(2, 4)] = [(f"proj{r}_0", gen_proj(r, 0))]
        appends[(4, 4)] = [(f"proj{r}_1", gen_proj(r, 1))]
        appends[(6, 4)] = [(f"proj{r}_2", gen_proj(r, 2))]
        blocks = [(0, 0, 0), (0, 1, 0), (0, 0, 1), (0, 1, 1),
                  (1, 0, 0), (1, 1, 0), (1, 0, 1), (1, 1, 1)]
        for bi, (bb, h, qb) in enumerate(blocks):
            if bi + 1 < len(blocks):
                nb, _, nq = blocks[bi + 1]
                next_q = [_nm_qk(r, nb, 0, 2 * nq), _nm_qk(r, nb, 0, 2 * nq + 1)]
            elif r + 1 < reps:
                next_q = [_nm_qk(r + 1, 0, 0, 0), _nm_qk(r + 1, 0, 0, 1)]
            else:
                next_q = None
            attn_block(r, bb, h, qb, appends, bi, next_q=next_q)
            if bi in (0, 2, 4):
                stage(r, bi // 2, 0)
            elif bi == 6:
                stage(r, 3, 0)
                stage(r, 4, 0)
            elif bi in (1, 3, 5):
                stage(r, bi // 2, 1)
                collect(r, bi // 2)
                if bi == 5 and r + 1 < reps:
                    add_qkv_batch(r + 1, 0)
            elif bi == 7:
                stage(r, 3, 1)
                collect(r, 3)
                stage(r, 4, 1)
                collect(r, 4)
        if r == reps - 1:
            sch.add(f"proj{r}_3", gen_proj(r, 3))
            sch.add(f"proj{r}_4", gen_proj(r, 4))
            sch.drain_all()


def _build(reps=1, collective=True, num_devices=N_CORES):
    nc = bacc.Bacc("TRN2", target_bir_lowering=False, debug=False,
                   num_devices=num_devices)
    # inputs are host-pre-tiled so every DMA reads one contiguous block;
    # x^T arrives in token-half pieces so cold QKV can start early
    xT = nc.dram_tensor("xT", [B, 2, 8, 128, N // 2], BF16,
                        kind="ExternalInput").ap()  # [bb, half, kc, p, tok]
    wqkvT = nc.dram_tensor("wqkvT", [128, 8, 3 * FPC], BF16,
                           kind="ExternalInput").ap()   # [p, kc, feat]
    bqk = nc.dram_tensor("bqk", [2, FPC, 1], F32, kind="ExternalInput").ap()
    wpT = nc.dram_tensor("wpT", [128, 8, FPC], BF16,
                         kind="ExternalInput").ap()     # [p, kc, fo]
    bp = nc.dram_tensor("bp", [FPC, 1], F32, kind="ExternalInput").ap()
    y = nc.dram_tensor("y", [FPC, T], F32, kind="ExternalOutput").ap()

    TPB = N // 128

    with tile.TileContext(nc) as tc:
        with tc.tile_pool(name="const", bufs=1) as const, \
             tc.tile_pool(name="slabs", bufs=1) as slabs, \
             tc.tile_pool(name="psSC", bufs=2, space="PSUM") as psSC, \
             tc.tile_pool(name="psAO", bufs=1, space="PSUM") as psAO, \
             tc.tile_pool(name="psF", bufs=2, space="PSUM") as psF, \
             tc.tile_pool(name="attnp", bufs=7) as attnp, \
             tc.tile_pool(name="aoraw", bufs=4) as aoraw, \
             tc.tile_pool(name="normp", bufs=3) as normp, \
             tc.tile_pool(name="rhp", bufs=4) as rhp, \
             tc.tile_pool(name="yp", bufs=2) as yp, \
             tc.tile_pool(name="dram", bufs=1, space="DRAM") as dram, \
             tc.tile_pool(name="dramloc", bufs=1, space="DRAM") as dramloc:
            C = _Ctx(nc, collective)
            C.psSC, C.psAO, C.psF = psSC, psAO, psF
            C.attnp, C.aoraw, C.normp, C.rhp, C.yp = (attnp, aoraw, normp,
                                                      rhp, yp)
            C.y = y
            C.sch = _Sched()

            # per-kc weight slices interleaved with batch-0 x^T halves so the
            # cold kc-outer QKV wave starts on the first arrivals
            w_all = const.tile([128, 8, 3 * FPC], BF16, tag="w", name="w_all")
            C.w_sb = [w_all[:, kc, :] for kc in range(8)]
            bqk_sb = const.tile([FPC, 2], F32, tag="bqk", name="bqk_sb")
            nc.sync.dma_start(out=bqk_sb[:, 0:1], in_=bqk[0])
            nc.sync.dma_start(out=bqk_sb[:, 1:2], in_=bqk[1])
            C.bqk_sb = bqk_sb
            xT_sb = [[None] * B for _ in range(8)]
            for bb in range(B):
                for kc in range(8):
                    t = const.tile([128, N], BF16, tag=f"xT{kc}_{bb}",
                                   name=f"xT{kc}_{bb}")
                    xT_sb[kc][bb] = t
            for kc in range(8):
                nc.sync.dma_start(out=w_all[:, kc, :], in_=wqkvT[:, kc, :])
                nc.sync.dma_start(out=xT_sb[kc][0][:, 0:1024],
                                  in_=xT[0, 0, kc])
            for bb in range(B):
                for hf in range(2):
                    if bb == 0 and hf == 0:
                        continue
                    for kc in range(8):
                        nc.sync.dma_start(
                            out=xT_sb[kc][bb][:, hf * 1024:(hf + 1) * 1024],
                            in_=xT[bb, hf, kc])
            C.xT_sb = xT_sb
            # proj weights are needed late; lowest DMA priority
            wp_all = const.tile([128, 8, FPC], BF16, tag="wp", name="wp_all")
            nc.sync.dma_start(out=wp_all[:], in_=wpT[:])
            C.wp_sb = [wp_all[:, kc, :] for kc in range(8)]
            bp_sb = const.tile([FPC, 1], F32, tag="bp", name="bp_sb")
            nc.sync.dma_start(out=bp_sb[:], in_=bp[:])
            C.bp_sb = bp_sb
            ident = const.tile([128, 128], BF16, tag="ident", name="ident")
            make_identity(nc, ident[:])
            C.ident = ident
            # ACT exp-table warm-up: a dummy exp during the input-DMA
            # prologue pulls the one-time ~2.7us ACT_TABLE_LOAD off the
            # first real attention exp.  Its output lands in y[0:1, 0:8],
            # which every projection block later overwrites (the WAW dep
            # keeps ordering correct).
            warm = const.tile([1, 8], F32, tag="warm", name="warm")
            nc.gpsimd.memset(warm[:], 0.0)
            warm2 = const.tile([1, 8], F32, tag="warm2", name="warm2")
            nc.scalar.activation(out=warm2[:], in_=warm[:],
                                 func=mybir.ActivationFunctionType.Exp)
            nc.sync.dma_start(out=y[0:1, 0:8], in_=warm2[:])

            # per-batch slabs: q/k transposed [feat, tok] (rows 0-63 head A,
            # 64-127 head B); v_ext [tok%128, tok_tile, head, HD+1] with ones
            # column at HD so attn@v_ext also yields softmax row sums.
            C.qsl = [slabs.tile([128, N], BF16, tag=f"qsl{b}", name=f"qsl{b}")
                     for b in range(B)]
            C.ksl = [slabs.tile([128, N], BF16, tag=f"ksl{b}", name=f"ksl{b}")
                     for b in range(B)]
            C.v_ext = [slabs.tile([128, TPB, HPC, HD + 1], BF16,
                                  tag=f"vext{b}", name=f"vext{b}")
                       for b in range(B)]
            C.aosl = [[slabs.tile([HD, N], BF16, tag=f"ao{b}{h}",
                                  name=f"ao{b}{h}")
                       for h in range(HPC)] for b in range(B)]
            for b in range(B):
                nc.gpsimd.memset(C.v_ext[b][:, :, :, HD:HD + 1], 1.0)

            C.ag_in, C.ag_out = {}, {}

            def alloc_ag(r):
                C.ag_in[r] = [dramloc.tile([FPC, w], BF16,
                                           name=f"agin{r}_{i}")
                              for i, (_, _, w) in enumerate(GBLK)]
                C.ag_out[r] = [dram.tile([DIM, w], BF16, addr_space="Shared",
                                         name=f"agout{r}_{i}")
                               for i, (_, _, w) in enumerate(GBLK)]
            C.alloc_ag = alloc_ag

            _emit_body(nc, tc, C, reps)
    nc.compile()
    return nc


def _prepare_in_maps(x, W_qkv, b_qkv, W_proj, b_proj):
    x = np.asarray(x, dtype=np.float32)
    W_qkv = np.asarray(W_qkv, dtype=np.float32)
    b_qkv = np.asarray(b_qkv, dtype=np.float32)
    W_proj = np.asarray(W_proj, dtype=np.float32)
    b_proj = np.asarray(b_proj, dtype=np.float32)

    xT = np.ascontiguousarray(x.reshape(T, DIM).T).astype(BF16_NP)
    # pre-tile to [bb, half, kc, 128, N//2] so device DMAs are contiguous
    xT = np.ascontiguousarray(
        xT.reshape(8, 128, B, 2, N // 2).transpose(2, 3, 0, 1, 4))
    # v bias folds through attention (softmax rows sum to 1) into the
    # projection bias: y += b_v @ W_proj.T
    bv = b_qkv[2 * DIM:3 * DIM]
    bp_eff = b_proj + bv @ W_proj.T

    in_maps = []
    for c in range(N_CORES):
        r0 = c * FPC
        wq = W_qkv[r0:r0 + FPC] * SCALE            # fold 1/sqrt(HD) into q
        wk = W_qkv[DIM + r0:DIM + r0 + FPC]
        wv = W_qkv[2 * DIM + r0:2 * DIM + r0 + FPC]
        wqkvT = np.ascontiguousarray(
            np.concatenate([wq, wk, wv], axis=0).T).astype(BF16_NP)
        wqkvT = np.ascontiguousarray(
            wqkvT.reshape(8, 128, 3 * FPC).transpose(1, 0, 2))
        bqk = np.stack([b_qkv[r0:r0 + FPC] * SCALE,
                        b_qkv[DIM + r0:DIM + r0 + FPC]])[:, :, None]
        wpT = np.ascontiguousarray(W_proj[r0:r0 + FPC].T).astype(BF16_NP)
        wpT = np.ascontiguousarray(
            wpT.reshape(8, 128, FPC).transpose(1, 0, 2))
        bp = bp_eff[r0:r0 + FPC][:, None]
        in_maps.append({
            "xT": xT,
            "wqkvT": wqkvT,
            "bqk": np.ascontiguousarray(bqk, dtype=np.float32),
            "wpT": wpT,
            "bp": np.ascontiguousarray(bp, dtype=np.float32),
        })
    return in_maps


def _assemble(results):
    # per-core y is [128, T] = (this core's 128 output features) x tokens
    cols = [np.asarray(results[c]["y"], dtype=np.float32).T
            for c in range(N_CORES)]
    return np.concatenate(cols, axis=1).reshape(B, N, DIM)


def kernel(x, W_qkv, b_qkv, W_proj, b_proj):
    if "nc" not in _NC_CACHE:
        _NC_CACHE["nc"] = _build()
    nc = _NC_CACHE["nc"]
    in_maps = _prepare_in_maps(x, W_qkv, b_qkv, W_proj, b_proj)
    res = run_bass_kernel_spmd(nc, in_maps, core_ids=list(range(N_CORES)))
    return _assemble(res.results)
